# revision 30
# baseline (speedup 1.0000x reference)
"""Single-head causal attention on 8 TRN2 NeuronCores (Bass/Tile), v3.

Self-contained: kernel(**inputs) takes the full inputs, shards across the
8 cores internally, runs one SPMD Bass NEFF, and reassembles the output.

Sharding: batch (4) x sequence-half (2).  Query blocks of 128 rows are
interleaved round-robin between the two cores of a batch pair (core h owns
global blocks g with g % 2 == h) so causal work is balanced and the SPMD
instruction stream is identical on every core; per-core differences are
carried by input data only (xt shard, maskc scalar).

Device kernel (bf16 matmuls, f32 psum):
  - x^T arrives window-major ([128, 8 windows x 8 d-chunks x 512 cols]) so
    each projection window is one contiguous DMA; windows are spread over
    the SP, Activation and Pool DMA queues (first window split in half so
    TensorE starts at the DMA-latency floor).
  - K^T and V^T live in one [128, 4096] tile (K on partitions 0:64, V on
    64:128); Q^T on partitions 0:64 only, so S^T matmuls contract on
    partitions 0:64 with no operand duplication.
  - V chunks become [V|1] AV weights via PE transpose + DVE copy (DMA
    transposes get serialized behind xt windows on the shared HWDGE rings).
  - Attention runs in two query passes of 1024 columns (acc = [65, 1024]
    psum, row 64 accumulates the softmax denominator via the ones column).
    Per key chunk: exp(j) is emitted, then filler work (partner projections,
    V transposes), then S^T(j+1), then AV(j) -- so TensorE streams without
    waiting on ScalarE.  Causal masking on Pool (tri for own-half diagonal
    blocks, per-core scalar for partner-half); diagonal AV pieces are
    emitted last so the mask lands first.  All matmuls are PSUM
    bank-aligned and start=True pieces touch each bank exactly once (a
    second start resets the whole bank).  GPSIMD never touches PSUM (HW
    restriction); psum->sbuf copies run on DVE.
  - The kernel returns numerator rows (64) + denominator row; the final
    divide happens on host.
"""

import numpy as np
import ml_dtypes

import concourse.bacc as bacc
import concourse.mybir as mybir
from concourse.bass_utils import run_bass_kernel_spmd
from concourse.tile import TileContext
from concourse.masks import make_upper_triangular, make_identity

B, T, D, DH = 4, 4096, 1024, 64
N_CORES = 8
RLOC = T // 2            # local query rows per core
NBLK = RLOC // 128       # 16 local query blocks
NDC = D // 128           # 8 contraction chunks
W = 512                  # projection window columns
NW = T // W              # 8 xt windows (4 own + 4 partner)
PASS = 1024              # query columns per attention pass
BF16 = mybir.dt.bfloat16
F32 = mybir.dt.float32
AF = mybir.ActivationFunctionType
BF = ml_dtypes.bfloat16


def _build_nc():
    nc = bacc.Bacc("TRN2", target_bir_lowering=False, debug=False,
                   num_devices=N_CORES)
    xt = nc.declare_dram_parameter("xt", [128, NW * NDC * W], BF16,
                                   isOutput=False)
    wkvq = nc.declare_dram_parameter("wkvq", [128, NDC * 192], BF16,
                                     isOutput=False)
    maskc = nc.declare_dram_parameter("maskc", [128, 1], F32, isOutput=False)
    out = nc.declare_dram_parameter("out", [DH + 1, RLOC], F32, isOutput=True)

    with TileContext(nc) as tc:
        with (
            tc.tile_pool(name="res", bufs=1) as res,
            tc.tile_pool(name="sb", bufs=2) as sb,
            tc.tile_pool(name="accp", bufs=1, space="PSUM") as accp,
            tc.tile_pool(name="ps", bufs=2, space="PSUM") as ps,
        ):
            xts = res.tile([128, NW * NDC * W], BF16)
            kvt = res.tile([128, T], BF16)       # rows 0:64 K^T, 64:128 V^T
            qt = res.tile([64, RLOC], BF16)
            vone = res.tile([128, 32 * (DH + 1)], BF16)
            wsb = res.tile([128, NDC * 192], BF16)
            osb = res.tile([DH + 1, RLOC], F32)
            tri = res.tile([128, 128], BF16)
            mc = res.tile([128, 1], F32)
            identb = res.tile([128, 64], BF16)

            # constants (Pool)
            make_upper_triangular(nc, tri[:, :], val=1.0, diag=True)
            make_identity(nc, identb[64:128, :])
            nc.vector.memset(vone[:, :], 1.0)

            def xt_dma(eng, w):
                eng.dma_start(
                    out=xts[:, w * NDC * W:(w + 1) * NDC * W],
                    in_=xt[:, w * NDC * W:(w + 1) * NDC * W])

            # first window in two halves so projections start sooner
            nc.scalar.dma_start(out=xts[:, 0:4 * W], in_=xt[:, 0:4 * W])
            nc.scalar.dma_start(out=xts[:, 4 * W:NDC * W],
                                in_=xt[:, 4 * W:NDC * W])

            # DMA queue assignments (order within each queue = emission
            # order; chosen so nothing waits on data produced later):
            #   Act:  xt w0, dummy, vtrA0-3, exps... (+vtrB hooks)
            #   SP:   wsb, xt w1, w2, mc, vtrA4-7, w3, w6, w7, vtrA8-15,
            #         out[0:1024], vtrB8-15, out[1024:1792], out[1792:2048]
            #   Pool: tri, xt w4, w5, masks/copies/epilogues
            nc.sync.dma_start(out=wsb[:, :], in_=wkvq[:, :])
            xt_dma(nc.sync, 1)
            xt_dma(nc.sync, 2)
            xt_dma(nc.gpsimd, 4)
            xt_dma(nc.gpsimd, 5)
            nc.gpsimd.dma_start(out=mc[:, :], in_=maskc[:, :])

            # copy engine per projection window (kvt copy, qt copy)
            # NOTE: GPSIMD cannot access PSUM on hardware -- psum->sbuf
            # copies must run on DVE (or ScalarE)
            COPY_ENG = {0: ("v", "v"), 1: ("v", "v"), 2: ("v", "v"),
                        3: ("v", "v"), 4: ("v", None), 5: ("v", None),
                        6: ("v", None), 7: ("v", None)}

            def ceng(key):
                return nc.vector if key == "v" else nc.gpsimd

            def proj_window(w, q_proj):
                """(Q +) KV projection matmuls for xt window w + psum->sbuf
                copies.  Q first so its copy unblocks S^T sooner."""
                ke, qe = COPY_ENG[w]
                if q_proj:
                    pq = ps.tile([64, W], F32, tag="pj")
                    for c in range(NDC):
                        nc.tensor.matmul(
                            pq[:, :],
                            wsb[:, c * 192 + 128:c * 192 + 192],
                            xts[:, (w * NDC + c) * W:(w * NDC + c + 1) * W],
                            start=(c == 0), stop=(c == NDC - 1))
                    ceng(qe).tensor_copy(qt[:, w * W:(w + 1) * W], pq[0:64, :])
                pkv = ps.tile([128, W], F32, tag="pj")
                for c in range(NDC):
                    nc.tensor.matmul(
                        pkv[:, :],
                        wsb[:, c * 192:c * 192 + 128],
                        xts[:, (w * NDC + c) * W:(w * NDC + c + 1) * W],
                        start=(c == 0), stop=(c == NDC - 1))
                if w == 0:
                    # split so chunk 0 (S^T lhsT + V transpose) unblocks early
                    ceng(ke).tensor_copy(kvt[:, 0:128], pkv[:, 0:128])
                    ceng(ke).tensor_copy(kvt[:, 128:W], pkv[:, 128:W])
                else:
                    ceng(ke).tensor_copy(kvt[:, w * W:(w + 1) * W], pkv[:, :])

            def vtr(s):
                """PE-transpose V chunk s into vone slot s (DVE copy out)."""
                ptr = ps.tile([128, DH], BF16, tag="pj")
                nc.tensor.transpose(ptr[:, :],
                                    kvt[64:128, s * 128:(s + 1) * 128],
                                    identb[64:128, :])
                nc.vector.tensor_copy(
                    vone[:, s * (DH + 1):s * (DH + 1) + DH], ptr[:, :])

            proj_window(0, True)
            for s in range(0, 4):
                vtr(s)

            # ---- attention ----
            def make_jobs(p):
                jobs = []
                base = PASS * p
                order = [("own", l) for l in range(NBLK)] + \
                    [("partner", l) for l in range(NBLK)]
                if p == 1:
                    order = ([("own", l) for l in range(8)] +
                             [("partner", l) for l in range(8)] +
                             [("own", l) for l in range(8, NBLK)] +
                             [("partner", l) for l in range(8, NBLK)])
                for kind, l in order:
                    if True:
                        c0 = max(128 * l, base)
                        c1 = base + PASS
                        if c0 >= c1:
                            continue
                        wins = [(c0, c1)]
                        if p == 0 and kind == "own" and c0 < 512:
                            wins = [(c0, 512), (512, c1)]
                        for w0, w1 in wins:
                            jobs.append(dict(
                                kind=kind, l=l, c0=w0, n=w1 - w0,
                                diag=(w0 == 128 * l),
                                slot=(l if kind == "own" else 16 + l),
                                kcol=(128 * l if kind == "own"
                                      else T // 2 + 128 * l)))
                return jobs

            def emit_st(job):
                pst = ps.tile([128, PASS], F32, tag="st")
                job["pst"] = pst
                n = job["n"]
                kc = job["kcol"]
                for m0 in range(0, n, W):
                    mn = min(W, n - m0)
                    nc.tensor.matmul(
                        pst[:, m0:m0 + mn],
                        kvt[0:64, kc:kc + 128],
                        qt[:, job["c0"] + m0:job["c0"] + m0 + mn],
                        start=True, stop=True, skip_group_check=True)

            def emit_exp(job):
                wt = sb.tile([128, PASS], BF16, tag="wt", bufs=4)
                job["wt"] = wt
                n = job["n"]
                nc.scalar.activation(wt[:, 0:n], job["pst"][:, 0:n], AF.Exp,
                                     scale=0.125)
                if job["diag"]:
                    if job["kind"] == "own":
                        nc.gpsimd.tensor_tensor(wt[:, 0:128], wt[:, 0:128],
                                                tri[:, :],
                                                mybir.AluOpType.mult)
                    else:
                        nc.gpsimd.tensor_scalar_mul(wt[:, 0:128], wt[:, 0:128],
                                                    mc[:, 0:1])

            def emit_av(job, acc, base, first):
                wt = job["wt"]
                c0, n = job["c0"], job["n"]
                a0, a1 = c0 - base, c0 - base + n
                # bank-aligned pieces; diagonal 128-col piece goes last so the
                # Pool mask write lands before PE reads it
                bounds = sorted({a0, a1} | {b for b in (512,) if a0 < b < a1})
                pieces = list(zip(bounds[:-1], bounds[1:]))
                if job["diag"] and n > 128 and not first:
                    d1 = a0 + 128
                    rest = []
                    for p0, p1 in pieces:
                        if p0 < d1:
                            if d1 < p1:
                                rest.append((d1, p1))
                        else:
                            rest.append((p0, p1))
                    pieces = rest + [(a0, d1)]
                for p0, p1 in pieces:
                    nc.tensor.matmul(
                        acc[:, p0:p1],
                        vone[:, job["slot"] * (DH + 1):
                             (job["slot"] + 1) * (DH + 1)],
                        wt[:, p0 - a0:p1 - a0],
                        start=first, stop=False, skip_group_check=True)

            def run_pass(p, hooks):
                base = PASS * p
                acc = accp.tile([DH + 1, PASS], F32, tag="acc")
                jobs = make_jobs(p)
                for i, job in enumerate(jobs):
                    if i == 0:
                        emit_st(jobs[0])
                    emit_exp(job)
                    for hook_i, fn in hooks:
                        if hook_i == i:
                            fn()
                    if i + 1 < len(jobs):
                        emit_st(jobs[i + 1])
                    emit_av(job, acc, base,
                            first=(job["kind"] == "own" and job["l"] == 0))
                    if job["kind"] == "partner":
                        m = job["l"]
                        if base <= 128 * m:
                            nc.vector.tensor_copy(
                                osb[:, m * 128:(m + 1) * 128],
                                acc[0:DH + 1, m * 128 - base:
                                    (m + 1) * 128 - base])

            # pass 0 jobs: own l0a,l0b,l1a,l1b,l2a,l2b,l3a,l3b,l4..l7 (0..11),
            # partner j0..j7 (12..19)
            hooks0 = [
                (0, lambda: proj_window(1, True)),
                (1, lambda: [vtr(s) for s in range(4, 8)]),
                (2, lambda: proj_window(4, False)),
                (3, lambda: [vtr(s) for s in range(16, 20)]),
                (5, lambda: proj_window(5, False)),
                (6, lambda: [vtr(s) for s in range(20, 24)]),
                (7, lambda: xt_dma(nc.sync, 3)),
                (8, lambda: xt_dma(nc.sync, 6)),
                (9, lambda: xt_dma(nc.sync, 7)),
                (13, lambda: proj_window(2, True)),
                (14, lambda: [vtr(s) for s in range(8, 12)]),
                (15, lambda: proj_window(3, True)),
                (16, lambda: [vtr(s) for s in range(12, 16)]),
            ]
            run_pass(0, hooks0)

            nc.sync.dma_start(out=out[:, 0:PASS], in_=osb[:, 0:PASS])

            hooks1 = [
                (0, lambda: proj_window(6, False)),
                (1, lambda: [vtr(s) for s in range(24, 28)]),
                (2, lambda: proj_window(7, False)),
                (3, lambda: [vtr(s) for s in range(28, 32)]),
                # after exp(30) the epilogue copies for blocks 8..13 exist
                (30, lambda: nc.sync.dma_start(
                    out=out[:, PASS:PASS + 768],
                    in_=osb[:, PASS:PASS + 768])),
            ]
            run_pass(1, hooks1)
            nc.sync.dma_start(out=out[:, PASS + 768:RLOC],
                              in_=osb[:, PASS + 768:RLOC])
    nc.compile()
    return nc


def _in_maps(x, Wk, Wq, Wv):
    wall = np.concatenate([Wk, Wv, Wq], axis=1)  # [1024, 192]
    wkvq_np = np.ascontiguousarray(
        wall.reshape(NDC, 128, 192).transpose(1, 0, 2).reshape(128, NDC * 192)
    ).astype(BF)
    in_maps = []
    for core in range(N_CORES):
        b, h = core // 2, core % 2
        own = [2 * l + h for l in range(NBLK)]
        other = [2 * l + (1 - h) for l in range(NBLK)]
        rows = np.concatenate(
            [x[b, g * 128:(g + 1) * 128, :] for g in own + other], 0)
        # rows: [4096, 1024] -> [p=128, w=8, c=8, j=512]
        xr = rows.reshape(NW, W, NDC, 128).transpose(3, 0, 2, 1)
        in_maps.append({
            "xt": np.ascontiguousarray(xr.reshape(128, NW * NDC * W)).astype(BF),
            "wkvq": wkvq_np,
            "maskc": np.full((128, 1), float(h), np.float32),
        })
    return in_maps


_NC = None


def kernel(x, Wk, Wq, Wv):
    global _NC
    x = np.asarray(x)
    Wk, Wq, Wv = np.asarray(Wk), np.asarray(Wq), np.asarray(Wv)
    if _NC is None:
        _NC = _build_nc()
    in_maps = _in_maps(x, Wk, Wq, Wv)
    res = run_bass_kernel_spmd(_NC, in_maps, core_ids=list(range(N_CORES)))
    out = np.empty((B, T, DH), np.float32)
    for core in range(N_CORES):
        b, h = core // 2, core % 2
        o = res.results[core]["out"]          # [65, 2048]
        vals = (o[0:DH, :] / o[DH:DH + 1, :]).T   # [2048, 64]
        for l in range(NBLK):
            g = 2 * l + h
            out[b, g * 128:(g + 1) * 128, :] = vals[l * 128:(l + 1) * 128, :]
    return out


# revision 32
# speedup vs baseline: 1.0434x; 1.0434x over previous
"""Single-head causal attention on 8 TRN2 NeuronCores (Bass/Tile), v3.

Self-contained: kernel(**inputs) takes the full inputs, shards across the
8 cores internally, runs one SPMD Bass NEFF, and reassembles the output.

Sharding: batch (4) x sequence-half (2).  Query blocks of 128 rows are
interleaved round-robin between the two cores of a batch pair (core h owns
global blocks g with g % 2 == h) so causal work is balanced and the SPMD
instruction stream is identical on every core; per-core differences are
carried by input data only (xt shard, maskc scalar).

Device kernel (bf16 matmuls, f32 psum):
  - x^T arrives window-major ([128, 8 windows x 8 d-chunks x 512 cols]) so
    each projection window is one contiguous DMA; windows are spread over
    the SP, Activation and Pool DMA queues (first window split in half so
    TensorE starts at the DMA-latency floor).
  - K^T and V^T live in one [128, 4096] tile (K on partitions 0:64, V on
    64:128); Q^T on partitions 0:64 only, so S^T matmuls contract on
    partitions 0:64 with no operand duplication.
  - V chunks become [V|1] AV weights via PE transpose + DVE copy (DMA
    transposes get serialized behind xt windows on the shared HWDGE rings).
  - Attention runs in two query passes of 1024 columns (acc = [65, 1024]
    psum, row 64 accumulates the softmax denominator via the ones column).
    Per key chunk: exp(j) is emitted, then filler work (partner projections,
    V transposes), then S^T(j+1), then AV(j) -- so TensorE streams without
    waiting on ScalarE.  Causal masking on Pool (tri for own-half diagonal
    blocks, per-core scalar for partner-half); diagonal AV pieces are
    emitted last so the mask lands first.  All matmuls are PSUM
    bank-aligned and start=True pieces touch each bank exactly once (a
    second start resets the whole bank).  GPSIMD never touches PSUM (HW
    restriction); psum->sbuf copies run on DVE.
  - The kernel returns numerator rows (64) + denominator row; the final
    divide happens on host.
"""

import numpy as np
import ml_dtypes

import concourse.bacc as bacc
import concourse.mybir as mybir
from concourse.bass_utils import run_bass_kernel_spmd
from concourse.tile import TileContext
from concourse.masks import make_upper_triangular, make_identity

B, T, D, DH = 4, 4096, 1024, 64
N_CORES = 8
RLOC = T // 2            # local query rows per core
NBLK = RLOC // 128       # 16 local query blocks
NDC = D // 128           # 8 contraction chunks
W = 512                  # projection window columns
NW = T // W              # 8 xt windows (4 own + 4 partner)
PASS = 1024              # query columns per attention pass
BF16 = mybir.dt.bfloat16
F32 = mybir.dt.float32
AF = mybir.ActivationFunctionType
BF = ml_dtypes.bfloat16


def _build_nc():
    nc = bacc.Bacc("TRN2", target_bir_lowering=False, debug=False,
                   num_devices=N_CORES)
    xt = nc.declare_dram_parameter("xt", [128, NW * NDC * W], BF16,
                                   isOutput=False)
    wkvq = nc.declare_dram_parameter("wkvq", [128, NDC * 192], BF16,
                                     isOutput=False)
    maskc = nc.declare_dram_parameter("maskc", [128, 1], F32, isOutput=False)
    out = nc.declare_dram_parameter("out", [DH + 1, RLOC], F32, isOutput=True)

    with TileContext(nc) as tc:
        with (
            tc.tile_pool(name="res", bufs=1) as res,
            tc.tile_pool(name="sb", bufs=2) as sb,
            tc.tile_pool(name="accp", bufs=1, space="PSUM") as accp,
            tc.tile_pool(name="ps", bufs=2, space="PSUM") as ps,
        ):
            xts = res.tile([128, NW * NDC * W], BF16)
            kvt = res.tile([128, T], BF16)       # rows 0:64 K^T, 64:128 V^T
            qt = res.tile([64, RLOC], BF16)
            vone = res.tile([128, 32 * (DH + 1)], BF16)
            wsb = res.tile([128, NDC * 192], BF16)
            osb = res.tile([DH + 1, RLOC], F32)
            tri = res.tile([128, 128], BF16)
            mc = res.tile([128, 1], F32)
            identb = res.tile([128, 64], BF16)

            # constants (Pool)
            make_upper_triangular(nc, tri[:, :], val=1.0, diag=True)
            make_identity(nc, identb[64:128, :])
            nc.vector.memset(vone[:, :], 1.0)

            def xt_dma(eng, w):
                eng.dma_start(
                    out=xts[:, w * NDC * W:(w + 1) * NDC * W],
                    in_=xt[:, w * NDC * W:(w + 1) * NDC * W])

            # first window in two halves so projections start sooner
            nc.scalar.dma_start(out=xts[:, 0:4 * W], in_=xt[:, 0:4 * W])
            nc.scalar.dma_start(out=xts[:, 4 * W:NDC * W],
                                in_=xt[:, 4 * W:NDC * W])

            # DMA queue assignments (order within each queue = emission
            # order; chosen so nothing waits on data produced later):
            #   Act:  xt w0, dummy, vtrA0-3, exps... (+vtrB hooks)
            #   SP:   wsb, xt w1, w2, mc, vtrA4-7, w3, w6, w7, vtrA8-15,
            #         out[0:1024], vtrB8-15, out[1024:1792], out[1792:2048]
            #   Pool: tri, xt w4, w5, masks/copies/epilogues
            nc.sync.dma_start(out=wsb[:, :], in_=wkvq[:, :])
            xt_dma(nc.sync, 1)
            xt_dma(nc.sync, 2)
            xt_dma(nc.gpsimd, 4)
            xt_dma(nc.gpsimd, 5)
            nc.gpsimd.dma_start(out=mc[:, :], in_=maskc[:, :])

            # copy engine per projection window (kvt copy, qt copy)
            # NOTE: GPSIMD cannot access PSUM on hardware -- psum->sbuf
            # copies must run on DVE (or ScalarE)
            COPY_ENG = {0: ("v", "v"), 1: ("v", "v"), 2: ("v", "v"),
                        3: ("v", "v"), 4: ("v", None), 5: ("v", None),
                        6: ("v", None), 7: ("v", None)}

            def ceng(key):
                return nc.vector if key == "v" else nc.gpsimd

            def proj_window(w, q_proj):
                """(Q +) KV projection matmuls for xt window w + psum->sbuf
                copies.  Q first so its copy unblocks S^T sooner."""
                ke, qe = COPY_ENG[w]
                if q_proj:
                    pq = ps.tile([64, W], F32, tag="pj")
                    for c in range(NDC):
                        nc.tensor.matmul(
                            pq[:, :],
                            wsb[:, c * 192 + 128:c * 192 + 192],
                            xts[:, (w * NDC + c) * W:(w * NDC + c + 1) * W],
                            start=(c == 0), stop=(c == NDC - 1))
                    ceng(qe).tensor_copy(qt[:, w * W:(w + 1) * W], pq[0:64, :])
                pkv = ps.tile([128, W], F32, tag="pj")
                for c in range(NDC):
                    nc.tensor.matmul(
                        pkv[:, :],
                        wsb[:, c * 192:c * 192 + 128],
                        xts[:, (w * NDC + c) * W:(w * NDC + c + 1) * W],
                        start=(c == 0), stop=(c == NDC - 1))
                if w == 0:
                    # split so chunk 0 (S^T lhsT + V transpose) unblocks early
                    ceng(ke).tensor_copy(kvt[:, 0:128], pkv[:, 0:128])
                    ceng(ke).tensor_copy(kvt[:, 128:W], pkv[:, 128:W])
                else:
                    ceng(ke).tensor_copy(kvt[:, w * W:(w + 1) * W], pkv[:, :])

            def vtr(s):
                """PE-transpose V chunk s into vone slot s (DVE copy out)."""
                ptr = ps.tile([128, DH], BF16, tag="pj")
                nc.tensor.transpose(ptr[:, :],
                                    kvt[64:128, s * 128:(s + 1) * 128],
                                    identb[64:128, :])
                nc.vector.tensor_copy(
                    vone[:, s * (DH + 1):s * (DH + 1) + DH], ptr[:, :])

            proj_window(0, True)
            for s in range(0, 4):
                vtr(s)

            # ---- attention ----
            def make_jobs(p):
                jobs = []
                base = PASS * p
                order = [("own", l) for l in range(NBLK)] + \
                    [("partner", l) for l in range(NBLK)]
                if p == 1:
                    order = ([("own", l) for l in range(8)] +
                             [("partner", l) for l in range(8)] +
                             [("own", l) for l in range(8, NBLK)] +
                             [("partner", l) for l in range(8, NBLK)])
                for kind, l in order:
                    if True:
                        c0 = max(128 * l, base)
                        c1 = base + PASS
                        if c0 >= c1:
                            continue
                        wins = [(c0, c1)]
                        if p == 0 and kind == "own" and c0 < 512:
                            wins = [(c0, 512), (512, c1)]
                        for w0, w1 in wins:
                            jobs.append(dict(
                                kind=kind, l=l, c0=w0, n=w1 - w0,
                                diag=(w0 == 128 * l),
                                slot=(l if kind == "own" else 16 + l),
                                kcol=(128 * l if kind == "own"
                                      else T // 2 + 128 * l)))
                return jobs

            def emit_st(job):
                pst = ps.tile([128, PASS], F32, tag="st")
                job["pst"] = pst
                n = job["n"]
                kc = job["kcol"]
                for m0 in range(0, n, W):
                    mn = min(W, n - m0)
                    nc.tensor.matmul(
                        pst[:, m0:m0 + mn],
                        kvt[0:64, kc:kc + 128],
                        qt[:, job["c0"] + m0:job["c0"] + m0 + mn],
                        start=True, stop=True, skip_group_check=True)

            def emit_exp(job):
                wt = sb.tile([128, PASS], BF16, tag="wt", bufs=4)
                job["wt"] = wt
                n = job["n"]
                nc.scalar.activation(wt[:, 0:n], job["pst"][:, 0:n], AF.Exp,
                                     scale=0.125)
                if job["diag"]:
                    if job["kind"] == "own":
                        nc.gpsimd.tensor_tensor(wt[:, 0:128], wt[:, 0:128],
                                                tri[:, :],
                                                mybir.AluOpType.mult)
                    else:
                        nc.gpsimd.tensor_scalar_mul(wt[:, 0:128], wt[:, 0:128],
                                                    mc[:, 0:1])

            def emit_av(job, acc, base, first):
                wt = job["wt"]
                c0, n = job["c0"], job["n"]
                a0, a1 = c0 - base, c0 - base + n
                # bank-aligned pieces; diagonal 128-col piece goes last so the
                # Pool mask write lands before PE reads it
                bounds = sorted({a0, a1} | {b for b in (512,) if a0 < b < a1})
                pieces = list(zip(bounds[:-1], bounds[1:]))
                if job["diag"] and n > 128 and not first:
                    d1 = a0 + 128
                    rest = []
                    for p0, p1 in pieces:
                        if p0 < d1:
                            if d1 < p1:
                                rest.append((d1, p1))
                        else:
                            rest.append((p0, p1))
                    pieces = rest + [(a0, d1)]
                for p0, p1 in pieces:
                    nc.tensor.matmul(
                        acc[:, p0:p1],
                        vone[:, job["slot"] * (DH + 1):
                             (job["slot"] + 1) * (DH + 1)],
                        wt[:, p0 - a0:p1 - a0],
                        start=first, stop=False, skip_group_check=True)

            def run_pass(p, hooks):
                base = PASS * p
                acc = accp.tile([DH + 1, PASS], F32, tag="acc")
                jobs = make_jobs(p)
                for i, job in enumerate(jobs):
                    if i == 0:
                        emit_st(jobs[0])
                    emit_exp(job)
                    for hook_i, fn in hooks:
                        if hook_i == i:
                            fn()
                    if i + 1 < len(jobs):
                        emit_st(jobs[i + 1])
                    emit_av(job, acc, base,
                            first=(job["kind"] == "own" and job["l"] == 0))
                    if job["kind"] == "partner":
                        m = job["l"]
                        if base <= 128 * m:
                            nc.vector.tensor_copy(
                                osb[:, m * 128:(m + 1) * 128],
                                acc[0:DH + 1, m * 128 - base:
                                    (m + 1) * 128 - base])

            # pass 0 jobs: own l0a,l0b,l1a,l1b,l2a,l2b,l3a,l3b,l4..l7 (0..11),
            # partner j0..j7 (12..19)
            hooks0 = [
                (0, lambda: proj_window(1, True)),
                (1, lambda: [vtr(s) for s in range(4, 8)]),
                (2, lambda: proj_window(4, False)),
                (3, lambda: [vtr(s) for s in range(16, 20)]),
                (5, lambda: proj_window(5, False)),
                (6, lambda: [vtr(s) for s in range(20, 24)]),
                (7, lambda: xt_dma(nc.sync, 3)),
                (8, lambda: xt_dma(nc.sync, 6)),
                (9, lambda: xt_dma(nc.sync, 7)),
                (15, lambda: proj_window(2, True)),
                (16, lambda: [vtr(s) for s in range(8, 12)]),
                (17, lambda: proj_window(3, True)),
                (18, lambda: [vtr(s) for s in range(12, 16)]),
            ]
            run_pass(0, hooks0)

            nc.sync.dma_start(out=out[:, 0:PASS], in_=osb[:, 0:PASS])

            hooks1 = [
                (16, lambda: proj_window(6, False)),
                (17, lambda: [vtr(s) for s in range(24, 28)]),
                (18, lambda: proj_window(7, False)),
                (19, lambda: [vtr(s) for s in range(28, 32)]),
                # after exp(30) the epilogue copies for blocks 8..13 exist
                (30, lambda: nc.sync.dma_start(
                    out=out[:, PASS:PASS + 768],
                    in_=osb[:, PASS:PASS + 768])),
            ]
            run_pass(1, hooks1)
            nc.sync.dma_start(out=out[:, PASS + 768:RLOC],
                              in_=osb[:, PASS + 768:RLOC])
    nc.compile()
    return nc


def _in_maps(x, Wk, Wq, Wv):
    wall = np.concatenate([Wk, Wv, Wq], axis=1)  # [1024, 192]
    wkvq_np = np.ascontiguousarray(
        wall.reshape(NDC, 128, 192).transpose(1, 0, 2).reshape(128, NDC * 192)
    ).astype(BF)
    in_maps = []
    for core in range(N_CORES):
        b, h = core // 2, core % 2
        own = [2 * l + h for l in range(NBLK)]
        other = [2 * l + (1 - h) for l in range(NBLK)]
        rows = np.concatenate(
            [x[b, g * 128:(g + 1) * 128, :] for g in own + other], 0)
        # rows: [4096, 1024] -> [p=128, w=8, c=8, j=512]
        xr = rows.reshape(NW, W, NDC, 128).transpose(3, 0, 2, 1)
        in_maps.append({
            "xt": np.ascontiguousarray(xr.reshape(128, NW * NDC * W)).astype(BF),
            "wkvq": wkvq_np,
            "maskc": np.full((128, 1), float(h), np.float32),
        })
    return in_maps


_NC = None


def kernel(x, Wk, Wq, Wv):
    global _NC
    x = np.asarray(x)
    Wk, Wq, Wv = np.asarray(Wk), np.asarray(Wq), np.asarray(Wv)
    if _NC is None:
        _NC = _build_nc()
    in_maps = _in_maps(x, Wk, Wq, Wv)
    res = run_bass_kernel_spmd(_NC, in_maps, core_ids=list(range(N_CORES)))
    out = np.empty((B, T, DH), np.float32)
    for core in range(N_CORES):
        b, h = core // 2, core % 2
        o = res.results[core]["out"]          # [65, 2048]
        vals = (o[0:DH, :] / o[DH:DH + 1, :]).T   # [2048, 64]
        for l in range(NBLK):
            g = 2 * l + h
            out[b, g * 128:(g + 1) * 128, :] = vals[l * 128:(l + 1) * 128, :]
    return out
